# revision 12
# baseline (speedup 1.0000x reference)
"""GQA attention prefill (B=2, S=1024, D=4096, H=32, KVH=8, HD=128) on 8 TRN2
NeuronCores.

Sharding: tensor-parallel over heads. Core c owns KV head c and query heads
4c..4c+3 (GQA groups align with cores), i.e. column-shards of wq/wk/wv and the
matching row-shard of wo. Each core computes its partial `attn_c @ wo_c`
([B*S, D]); the host sums the 8 partials.

Device inputs are host-pretiled into exact SBUF layouts so every DMA reads
fully contiguous per-partition lines (see _tile_* helpers):
  xh   [128, B*nkb, dc, 128]  bf16  activation half-chunks (contraction on
                                    partitions, 128 tokens per chunk)
  wq   [128, NH, dc, HD]  bf16  rope-permuted (even dims then odd, per head)
  wk   [128, dc, HD]      bf16  rope-permuted
  wv   [128, dc, HD]      bf16
  wo   [128, D/512, NH, 512] bf16
  sw   [128, 128]         bf16  identity (PE-transpose operand for V)
  c2   [128, S]   f32   rope cos table, duplicated across the two 64-row halves
  s2   [128, S]   f32   rope sin table, [-sin; +sin]
  mt   [128,128]  f32   causal diagonal-block additive mask * sqrt(HD)   (causal)
  mt   [S, S]     bf16  full transposed additive mask * sqrt(HD)         (general)

Q/K are produced transposed ([d, tok]) straight out of the projection matmul;
scores are computed transposed ([k, q]) so P^T feeds wide-N PV matmuls
(nested causal ranges) with no transposes anywhere. Softmax denominators are
built OFF the PE: a DVE pairwise add-tree folds the nkb k-blocks of P^T into
one [128, s] tile, a single GpSimd partition_all_reduce produces the
broadcast denominators, and a DVE reciprocal feeds the PV normalization
multiply. V is projected transposed and PE-transposed back. Rope's even/odd
pairing becomes a contiguous partition-half swap by permuting the weight
columns; the swap is two SBUF->SBUF DMAs. Softmax skips the max-subtraction
(scores are O(10); exp accumulates in fp32).

Startup: the first matmul needs only wk + the first x half-chunk. All x and
projection-weight DMAs ride the sync (HWDGE) queue in exact consumption order
(wk, x0, wv, x1, wq0, x2, wq1, x3, wq2, wq3, x4..), while the small rope/mask
tables load concurrently on the gpsimd queue — so the PE starts ~13us in and
never starves while the remaining weights stream. The first four half-chunks'
projection units are emitted in a hand-crafted order matched to DMA arrival.
wo rides the (otherwise idle) scalar queue so oproj never waits behind
output-store DMAs.

Scheduling: all work is emitted as unit-closures and "zipped" — batch 1's
projections interleave batch 0's attention, batch 0's output projection
interleaves batch 1's attention — so the in-order PE queue always holds dense
matmuls while exp/softmax chains drain on the other engines. PSUM evictions
are emitted one projection late to avoid head-of-line FIFO blocking.
"""

import math
from contextlib import ExitStack

import numpy as np
import ml_dtypes

import concourse.bass as bass
import concourse.bass_isa as bass_isa
import concourse.mybir as mybir
import concourse.tile as tile
from concourse import bacc
from concourse.bass_utils import run_bass_kernel_spmd

BF16 = mybir.dt.bfloat16
F32 = mybir.dt.float32
NPBF16 = ml_dtypes.bfloat16

B, S, D, H, KVH, HD = 2, 1024, 4096, 32, 8, 128
NCORES = 8
NH = H // NCORES          # 4 query heads per core
DC = D // 128             # 32 contraction chunks
SQ = 1.0 / math.sqrt(HD)


def _chunks(q0, qend, step=512):
    qs = q0
    while qs < qend:
        nq = min(step, qend - qs)
        yield qs, nq
        qs += nq


def build_program(causal, s=S, d=D):
    """Build the per-core SPMD program. s/d are overridable for sim tests."""
    dc = d // 128
    nkb = s // 128            # number of 128-wide key/query/token blocks per batch
    qcols = NH * HD

    # pT packing offsets: causal keeps only k-block ki's valid q range [128ki, s)
    if causal:
        q0s = [ki * 128 for ki in range(nkb)]
    else:
        q0s = [0] * nkb
    offs, acc = [], 0
    for ki in range(nkb):
        offs.append(acc)
        acc += s - q0s[ki]
    pt_len = acc

    nc = bacc.Bacc(
        "TRN2",
        target_bir_lowering=False,
        debug=False,
        enable_asserts=False,
        num_devices=1,
    )
    # all inputs are host-pretiled into the exact SBUF layouts, so every DMA
    # below reads fully contiguous per-partition lines
    xh = nc.dram_tensor("xh", [128, B * nkb, dc, 128], BF16, kind="ExternalInput").ap()
    wq = nc.dram_tensor("wq", [128, NH, dc, HD], BF16, kind="ExternalInput").ap()
    wk = nc.dram_tensor("wk", [128, dc, HD], BF16, kind="ExternalInput").ap()
    wv = nc.dram_tensor("wv", [128, dc, HD], BF16, kind="ExternalInput").ap()
    wo = nc.dram_tensor("wo", [128, d // 512, NH, 512], BF16, kind="ExternalInput").ap()
    sw = nc.dram_tensor("sw", [128, 128], BF16, kind="ExternalInput").ap()
    c2 = nc.dram_tensor("c2", [128, s], F32, kind="ExternalInput").ap()
    s2 = nc.dram_tensor("s2", [128, s], F32, kind="ExternalInput").ap()
    if causal:
        mt = nc.dram_tensor("mt", [128, 128], F32, kind="ExternalInput").ap()
    else:
        mt = nc.dram_tensor("mt", [s, s], BF16, kind="ExternalInput").ap()
    out = nc.dram_tensor("out", [B * s, d], BF16, kind="ExternalOutput").ap()

    with tile.TileContext(nc) as tc:
        with ExitStack() as ctx:
            const = ctx.enter_context(tc.tile_pool(name="const", bufs=1))
            xpool = ctx.enter_context(tc.tile_pool(name="xpool", bufs=4))
            wopool = ctx.enter_context(tc.tile_pool(name="wopool", bufs=2))
            qkv = ctx.enter_context(tc.tile_pool(name="qkv", bufs=2))
            ptp = ctx.enter_context(tc.tile_pool(name="ptp", bufs=2))
            rp = ctx.enter_context(tc.tile_pool(name="rp", bufs=3))
            tre = ctx.enter_context(tc.tile_pool(name="tre", bufs=2 if causal else 1))
            small = ctx.enter_context(tc.tile_pool(name="small", bufs=2))
            oev = ctx.enter_context(tc.tile_pool(name="oev", bufs=2))
            psm = ctx.enter_context(tc.tile_pool(name="psm", bufs=4, space="PSUM"))
            pss = ctx.enter_context(tc.tile_pool(name="pss", bufs=2, space="PSUM"))

            # wk rides the sync queue FIRST (in front of the x stream): the
            # opening K-projection chain needs exactly wk + the first x half.
            # Split in two so the first 16 accumulation matmuls can start a
            # hair earlier.
            wk_sb = const.tile([128, dc, HD], BF16)
            nc.sync.dma_start(wk_sb[:, 0 : dc // 2, :], wk[:, 0 : dc // 2, :])
            nc.sync.dma_start(wk_sb[:, dc // 2 :, :], wk[:, dc // 2 :, :])
            # small rope/mask tables load concurrently on the gpsimd queue
            id_sb = const.tile([128, 128], BF16)
            nc.gpsimd.dma_start(id_sb[:], sw[:])
            c2_sb = const.tile([128, s], F32)
            nc.gpsimd.dma_start(c2_sb[:], c2[:])
            s2_sb = const.tile([128, s], F32)
            nc.gpsimd.dma_start(s2_sb[:], s2[:])
            if causal:
                mt_sb = const.tile([128, 128], F32)
                nc.gpsimd.dma_start(mt_sb[:], mt[:])
            else:
                mt_sb = const.tile([128, nkb, s], BF16)
                nc.gpsimd.dma_start(mt_sb[:], mt.rearrange("(kb p) q -> p kb q", p=128))
            # wv/wq are loaded lazily inside the first units that need them,
            # interleaved into the sync queue in exact consumption order.
            wv_sb = const.tile([128, dc, HD], BF16)
            wq_sb = const.tile([128, NH, dc, HD], BF16)
            loaded = set()
            ones_sb = const.tile([128, 1], BF16)
            nc.vector.memset(ones_sb[:], 1.0)

            def rope(ps, tok0, w, out_slice):
                """ps: [128, w] psum with raw projected Q/K block (d-permuted).
                out = raw*c2 + halfswap(raw)*s2, written as bf16 to out_slice.
                Only the ACT eviction touches PSUM; the swap is two SBUF
                partition-block DMAs and the muls run from SBUF on gpsimd/DVE."""
                raw = rp.tile([128, w], BF16, tag="raw", name=f"raw_{tok0}")
                nc.scalar.copy(raw[:], ps[:, :w])
                swt = rp.tile([128, w], BF16, tag="swt", name=f"swt_{tok0}")
                nc.sync.dma_start(swt[0:64, :], raw[64:128, :])
                nc.sync.dma_start(swt[64:128, :], raw[0:64, :])
                t1 = rp.tile([128, w], F32, tag="t1", name=f"t1_{tok0}")
                nc.vector.tensor_mul(t1[:], swt[:], s2_sb[:, tok0 : tok0 + w])
                t2 = rp.tile([128, w], F32, tag="t2", name=f"t2_{tok0}")
                nc.vector.tensor_mul(t2[:], raw[:], c2_sb[:, tok0 : tok0 + w])
                nc.gpsimd.tensor_add(out_slice, t2[:], t1[:])

            def phase2(b):
                """Stream x half-chunks, project Q/K/V for batch b. Returns
                the per-batch activation tiles."""
                qT_sb = qkv.tile([128, NH, s], BF16, tag="qT", name=f"qT_{b}")
                kT_sb = qkv.tile([128, s], BF16, tag="kT", name=f"kT_{b}")
                vT_sb = qkv.tile([128, s], BF16, tag="vT", name=f"vT_{b}")
                v_sb = qkv.tile([128, nkb, HD], BF16, tag="v", name=f"v_{b}")
                attnT_sb = qkv.tile([128, NH, s], BF16, tag="attnT", name=f"attnT_{b}")

                # evictions/rope are emitted one projection late, so each
                # engine's FIFO only sees work whose PSUM inputs are (nearly)
                # ready — avoids head-of-line blocking behind matmul chains.
                pending = []

                def flush(keep):
                    while len(pending) > keep:
                        kind, ps, tok0_, w_ = pending.pop(0)
                        if kind == "k":
                            rope(ps, tok0_, w_, kT_sb[:, tok0_ : tok0_ + w_])
                        elif kind.startswith("q"):
                            h = int(kind[1:])
                            rope(ps, tok0_, w_, qT_sb[:, h, tok0_ : tok0_ + w_])
                        else:  # vt
                            nc.vector.tensor_copy(vT_sb[:, tok0_ : tok0_ + w_], ps[:])
                            kb = tok0_ // 128
                            vtp = pss.tile(
                                [128, HD], BF16, tag="sm", name=f"vtp_{b}_{kb}"
                            )
                            nc.tensor.transpose(
                                vtp[:], vT_sb[:, kb * 128 : (kb + 1) * 128], id_sb[:]
                            )
                            nc.scalar.copy(v_sb[:, kb, :], vtp[:])

                xcs = {}

                def mk_k(j):
                    def u():
                        xc = xpool.tile([128, dc, 128], BF16, tag="xc", name=f"xc_{b}_{j}")
                        nc.sync.dma_start(xc[:], xh[:, b * nkb + j, :, :])
                        xcs[j] = xc
                        k_ps = psm.tile([128, 128], F32, tag="mm", name=f"kps_{b}_{j}")
                        for c in range(dc):
                            nc.tensor.matmul(
                                k_ps[:],
                                wk_sb[:, c, :],
                                xcs[j][:, c, :],
                                start=(c == 0),
                                stop=(c == dc - 1),
                            )
                        pending.append(("k", k_ps, j * 128, 128))
                        flush(1)
                    return u

                def mk_vt(j):
                    def u():
                        # V projection, transposed like K (wide-N matmuls),
                        # then PE-transposed back to natural [tok, d] layout
                        if b == 0 and "wv" not in loaded:
                            loaded.add("wv")
                            nc.sync.dma_start(wv_sb[:], wv[:])
                        vt_ps = psm.tile([128, 128], F32, tag="mm", name=f"vtps_{b}_{j}")
                        for c in range(dc):
                            nc.tensor.matmul(
                                vt_ps[:],
                                wv_sb[:, c, :],
                                xcs[j][:, c, :],
                                start=(c == 0),
                                stop=(c == dc - 1),
                            )
                        pending.append(("vt", vt_ps, j * 128, 128))
                        flush(1)
                    return u

                def mk_q(j, h):
                    def u():
                        if b == 0 and f"wq{h}" not in loaded:
                            loaded.add(f"wq{h}")
                            nc.sync.dma_start(wq_sb[:, h, :, :], wq[:, h, :, :])
                        q_ps = psm.tile([128, 128], F32, tag="mm", name=f"qps_{b}_{j}_{h}")
                        for c in range(dc):
                            nc.tensor.matmul(
                                q_ps[:],
                                wq_sb[:, h, c, :],
                                xcs[j][:, c, :],
                                start=(c == 0),
                                stop=(c == dc - 1),
                            )
                        pending.append((f"q{h}", q_ps, j * 128, 128))
                        flush(1)
                    return u

                mk = {"k": mk_k, "v": mk_vt}

                units = []
                if b == 0 and nkb >= 4:
                    # hand-crafted order for the first four half-chunks,
                    # matched to the DMA arrival order on the sync queue:
                    # wk | x0 | wv | x1 | wq0 | x2 | wq1 | x3 | wq2 | wq3 | x4..
                    crafted = [
                        (0, "k"), (0, "v"), (1, "k"), (1, "v"),
                        (0, "q0"), (1, "q0"), (2, "k"), (2, "v"),
                        (0, "q1"), (1, "q1"), (2, "q0"), (3, "k"), (3, "v"),
                        (2, "q1"), (0, "q2"), (1, "q2"), (0, "q3"), (1, "q3"),
                        (2, "q2"), (3, "q0"), (3, "q1"), (2, "q3"),
                        (3, "q2"), (3, "q3"),
                    ]
                    for j, kind in crafted:
                        if kind.startswith("q"):
                            units.append(mk_q(j, int(kind[1:])))
                        else:
                            units.append(mk[kind](j))
                    first = 4
                else:
                    first = 0
                for j in range(first, nkb):
                    units.append(mk_k(j))
                    units.append(mk_vt(j))
                    for h in range(NH):
                        units.append(mk_q(j, h))
                units.append(lambda: flush(0))
                T = dict(qT=qT_sb, kT=kT_sb, vT=vT_sb, v=v_sb, attnT=attnT_sb)
                return T, units

            def attn(b, T):
                """Attention units for batch b, software-pipelined: PV of head
                h-1 is emitted after the scores of head h, so the softmax-
                denominator chain of head h-1 hides under head h's PE work."""
                qT_sb, kT_sb, v_sb, attnT_sb = T["qT"], T["kT"], T["v"], T["attnT"]
                stage1 = {}

                def attn_scores(h):
                    pT = ptp.tile([128, pt_len], BF16, tag="pt", name=f"pt_{b}_{h}")
                    for ki in range(nkb):
                        q0 = q0s[ki]
                        for qs_, nq in _chunks(q0, s):
                            sc = psm.tile([128, 512], F32, tag="mm", name=f"sc_{b}_{h}_{ki}_{qs_}")
                            nc.tensor.matmul(
                                sc[:, :nq],
                                kT_sb[:, ki * 128 : (ki + 1) * 128],
                                qT_sb[:, h, qs_ : qs_ + nq],
                                start=True,
                                stop=True,
                            )
                            if causal:
                                if qs_ == q0:  # diagonal block
                                    nc.vector.tensor_add(
                                        sc[:, 0:128], sc[:, 0:128], mt_sb[:]
                                    )
                            else:
                                nc.vector.tensor_add(
                                    sc[:, :nq], sc[:, :nq], mt_sb[:, ki, qs_ : qs_ + nq]
                                )
                            po = offs[ki] + qs_ - q0
                            nc.scalar.activation(
                                pT[:, po : po + nq],
                                sc[:, :nq],
                                mybir.ActivationFunctionType.Exp,
                                scale=SQ,
                            )

                    # Softmax denominators off the PE: DVE pairwise add-tree
                    # folds the nkb k-blocks of pT (nested causal q-ranges)
                    # into one [128, s] tile, then one GpSimd
                    # partition_all_reduce gives the broadcast denominators.
                    def pslice(ki, qa, qb):
                        return pT[:, offs[ki] + qa - q0s[ki] : offs[ki] + qb - q0s[ki]]

                    cur = [
                        (q0s[ki], (lambda ki_: lambda qa, qb: pslice(ki_, qa, qb))(ki))
                        for ki in range(nkb)
                    ]
                    level = 1
                    while len(cur) > 1:
                        nxt = []
                        for i in range(0, len(cur) - 1, 2):
                            (qa0, ga), (qb0, gb) = cur[i], cur[i + 1]
                            # intermediates are consumed within this head's
                            # tree; only the final fold is read later (by the
                            # ones-matmuls) and needs double-buffering
                            t = tre.tile(
                                [128, s - qa0], BF16, tag=f"f{level}_{i}",
                                bufs=(2 if len(cur) == 2 else 1) if causal else 1,
                                name=f"f{level}_{i}_{b}_{h}",
                            )
                            def gt(qa, qb, t=t, qa0=qa0):
                                return t[:, qa - qa0 : qb - qa0]
                            if qb0 > qa0:
                                nc.vector.tensor_copy(gt(qa0, qb0), ga(qa0, qb0))
                            nc.vector.tensor_add(gt(qb0, s), ga(qb0, s), gb(qb0, s))
                            nxt.append((qa0, gt))
                        if len(cur) % 2:
                            nxt.append(cur[-1])
                        cur = nxt
                        level += 1
                    return pT, cur[0][1]

                def attn_den(h):
                    # partition-reduce the folded S with a single short
                    # ones-matmul per 512 columns, then the baseline
                    # copy / broadcast / reciprocal chain. Emitted one head
                    # late (inside pv), so the ones-matmuls never sit at the
                    # head of the in-order PE queue waiting for the DVE tree.
                    pT, gS = stage1.pop(h)
                    nhalf = (s + 511) // 512
                    width = s // nhalf
                    sums = pss.tile([1, s], F32, tag="sums", bufs=1, name=f"sums_{b}_{h}")
                    rbrs = []
                    for hs in range(nhalf):
                        nc.tensor.matmul(
                            sums[0:1, hs * width : (hs + 1) * width],
                            ones_sb[:],
                            gS(hs * width, (hs + 1) * width),
                            start=True,
                            stop=True,
                            skip_group_check=True,
                        )
                        ssb = small.tile([1, width], F32, tag="ssb", bufs=4, name=f"ssb_{b}_{h}_{hs}")
                        nc.scalar.copy(ssb[0:1, :], sums[0:1, hs * width : (hs + 1) * width])
                        rb = small.tile([128, width], F32, tag="rb", bufs=4, name=f"rb_{b}_{h}_{hs}")
                        nc.gpsimd.partition_broadcast(rb[:], ssb[0:1, :])
                        rbr = small.tile([128, width], F32, tag="rbr", bufs=4, name=f"rbr_{b}_{h}_{hs}")
                        nc.vector.reciprocal_approx_fast(rbr[:], rb[:])
                        rbrs.append(rbr)
                    return pT, rbrs, width

                def attn_pv(h):
                    # wide-N PV: per q-chunk, each k-block contributes one
                    # matmul over its (nested) valid q range, accumulating in
                    # PSUM — ki==0 always covers the whole chunk, so it opens
                    # the group for every column.
                    pT, rbrs, width = attn_den(h)
                    for ci, (qs0, w) in enumerate(_chunks(0, s)):
                        o_ps = psm.tile([128, 512], F32, tag="mm", name=f"ops_{b}_{h}_{ci}")
                        kis = [
                            k for k in range(nkb) if (not causal) or q0s[k] < qs0 + w
                        ]
                        for j, ki in enumerate(kis):
                            qlo = max(q0s[ki], qs0)
                            nc.tensor.matmul(
                                o_ps[:, qlo - qs0 : w],
                                v_sb[:, ki, :],
                                pT[:, offs[ki] + qlo - q0s[ki] : offs[ki] + qs0 + w - q0s[ki]],
                                start=(j == 0),
                                stop=(j == len(kis) - 1),
                                skip_group_check=True,
                            )
                        nc.vector.tensor_mul(
                            attnT_sb[:, h, qs0 : qs0 + w],
                            o_ps[:, :w],
                            rbrs[qs0 // width][:, qs0 % width : qs0 % width + w],
                        )

                units = []
                for h in range(NH):
                    units.append(lambda h=h: stage1.__setitem__(h, attn_scores(h)))
                    if h > 0:
                        units.append(lambda h=h: attn_pv(h - 1))
                units.append(lambda: attn_pv(NH - 1))
                return units

            def oproj(b, T):
                """Output projection units (partial over this core's wo rows).
                wo loads ride the (idle) scalar queue so they never queue
                behind output-store DMAs. A one-element write gated on the
                batch's LAST qT rope pins each load's readiness past the
                startup DMA crunch — the Tile list scheduler would otherwise
                hoist these dependency-free loads to t=0 where they steal
                bandwidth from the opening wk/x transfers."""
                attnT_sb = T["attnT"]
                qT_sb = T["qT"]
                wo_nbs = {}

                def mk(nb, tp):
                    def u():
                        if tp == 0:
                            wo_nb = wopool.tile(
                                [128, NH, 512], BF16, tag="wo", name=f"wo_{b}_{nb}"
                            )
                            nc.vector.tensor_copy(
                                wo_nb[0:1, 0, 0:1], qT_sb[0:1, NH - 1, s - 1 : s]
                            )
                            nc.scalar.dma_start(wo_nb[:], wo[:, nb, :, :])
                            wo_nbs[nb] = wo_nb
                        ot = oev.tile([128, 2, 512], BF16, tag="ot", bufs=4, name=f"ot_{b}_{nb}_{tp}")
                        for half in range(2):
                            tbk = tp * 2 + half
                            o2 = psm.tile([128, 512], F32, tag="mm", name=f"o2_{b}_{nb}_{tbk}")
                            for h in range(NH):
                                nc.tensor.matmul(
                                    o2[:],
                                    attnT_sb[:, h, tbk * 128 : (tbk + 1) * 128],
                                    wo_nbs[nb][:, h, :],
                                    start=(h == 0),
                                    stop=(h == NH - 1),
                                )
                            if half == 0:
                                nc.scalar.copy(ot[:, half, :], o2[:])
                            else:
                                nc.vector.tensor_copy(ot[:, half, :], o2[:])
                        (nc.sync if tp % 2 == 0 else nc.gpsimd).dma_start(
                            out[
                                b * s + tp * 256 : b * s + (tp + 1) * 256,
                                nb * 512 : (nb + 1) * 512,
                            ].rearrange("(rh p) n -> p rh n", p=128),
                            ot[:],
                        )
                    return u

                return [mk(nb, tp) for nb in range(d // 512) for tp in range(nkb // 2)]

            def zip_emit(primary, filler):
                """Emit primary units with filler units woven between them, so
                the in-order PE queue always has dense matmul work to run
                while the primary's cross-engine chains (exp/softmax) drain."""
                k = max(1, len(filler) // max(1, len(primary)))
                fi = 0
                for i, u in enumerate(primary):
                    if i == len(primary) - 1:
                        # drain fillers before the last primary: its softmax
                        # chain holds PSUM slots and would starve them
                        while fi < len(filler):
                            filler[fi]()
                            fi += 1
                    u()
                    for _ in range(k):
                        if fi < len(filler):
                            filler[fi]()
                            fi += 1
                while fi < len(filler):
                    filler[fi]()
                    fi += 1

            # batch 0 projections run alone; batch 0 attention is interleaved
            # with batch 1 projections; batch 1 attention with batch 0 output
            # projection; batch 1 output projection drains at the end.
            T0, p20 = phase2(0)
            for u in p20:
                u()
            a0 = attn(0, T0)
            if B > 1:
                T1, p21 = phase2(1)
                zip_emit(a0, p21)
                a1 = attn(1, T1)
                o0 = oproj(0, T0)
                zip_emit(a1, o0)
                for u in oproj(1, T1):
                    u()
            else:
                for u in a0:
                    u()
                for u in oproj(0, T0):
                    u()
    nc.compile()
    return nc


# ---------------------------------------------------------------------------
# host side
# ---------------------------------------------------------------------------

_PERM = np.concatenate([np.arange(0, HD, 2), np.arange(1, HD, 2)])
_CACHE = {}


def _tile_xh(x, s=S, d=D):
    """[B, s, d] f32 -> [128, B*nkb, dc, 128] bf16 (SBUF half-chunk layout)."""
    nkb, dc = s // 128, d // 128
    t = x.reshape(B, nkb, 128, dc, 128).transpose(4, 0, 1, 3, 2)
    return np.ascontiguousarray(t.reshape(128, B * nkb, dc, 128)).astype(NPBF16)


def _tile_wq(w, d=D):
    """[d, NH*HD] f32 (already rope-permuted) -> [128, NH, dc, HD] bf16."""
    dc = d // 128
    t = w.reshape(dc, 128, NH, HD).transpose(1, 2, 0, 3)
    return np.ascontiguousarray(t).astype(NPBF16)


def _tile_wkv(w, d=D):
    """[d, HD] f32 -> [128, dc, HD] bf16."""
    dc = d // 128
    return np.ascontiguousarray(w.reshape(dc, 128, HD).transpose(1, 0, 2)).astype(NPBF16)


def _tile_wo(w, d=D):
    """[NH*HD, d] f32 -> [128, d//512, NH, 512] bf16."""
    t = w.reshape(NH, 128, d // 512, 512).transpose(1, 2, 0, 3)
    return np.ascontiguousarray(t).astype(NPBF16)


def _get_program(causal):
    if causal not in _CACHE:
        _CACHE[causal] = build_program(causal)
    return _CACHE[causal]


def _is_causal(mask):
    iu = np.triu_indices(S, 1)
    il = np.tril_indices(S)
    return bool(np.all(mask[il] == 0.0) and np.all(mask[iu] < -1e8))


def make_in_maps(x, cos, sin, mask, wq, wk, wv, wo, causal):
    x = np.asarray(x, dtype=np.float32)
    cos = np.asarray(cos, dtype=np.float32)
    sin = np.asarray(sin, dtype=np.float32)
    mask = np.asarray(mask, dtype=np.float32)
    wq = np.asarray(wq, dtype=np.float32)
    wk = np.asarray(wk, dtype=np.float32)
    wv = np.asarray(wv, dtype=np.float32)
    wo = np.asarray(wo, dtype=np.float32)

    xh = _tile_xh(x)
    c2 = np.ascontiguousarray(np.concatenate([cos.T, cos.T], 0)).astype(np.float32)
    s2 = np.ascontiguousarray(np.concatenate([-sin.T, sin.T], 0)).astype(np.float32)
    swm = np.eye(128, dtype=np.float32).astype(NPBF16)  # transpose identity
    if causal:
        mt = np.ascontiguousarray(mask[:128, :128].T * math.sqrt(HD)).astype(np.float32)
    else:
        mt = np.ascontiguousarray(mask.T * math.sqrt(HD)).astype(NPBF16)

    in_maps = []
    for c in range(NCORES):
        wq_c = wq[:, c * NH * HD : (c + 1) * NH * HD].reshape(D, NH, HD)[:, :, _PERM]
        wq_c = _tile_wq(wq_c.reshape(D, NH * HD))
        wk_c = _tile_wkv(np.ascontiguousarray(wk[:, c * HD : (c + 1) * HD][:, _PERM]))
        wv_c = _tile_wkv(np.ascontiguousarray(wv[:, c * HD : (c + 1) * HD]))
        wo_c = _tile_wo(np.ascontiguousarray(wo[c * NH * HD : (c + 1) * NH * HD, :]))
        in_maps.append(
            {
                "xh": xh,
                "wq": wq_c,
                "wk": wk_c,
                "wv": wv_c,
                "wo": wo_c,
                "sw": swm,
                "c2": c2,
                "s2": s2,
                "mt": mt,
            }
        )
    return in_maps


def run(in_maps, causal, **kwargs):
    nc = _get_program(causal)
    return run_bass_kernel_spmd(nc, in_maps, core_ids=list(range(NCORES)), **kwargs)


def kernel(x, start_pos, cos, sin, mask, wq, wk, wv, wo):
    mask = np.asarray(mask, dtype=np.float32)
    causal = _is_causal(mask)
    in_maps = make_in_maps(x, cos, sin, mask, wq, wk, wv, wo, causal)
    res = run(in_maps, causal)
    acc = np.zeros((B * S, D), dtype=np.float32)
    for c in range(NCORES):
        acc += np.asarray(res.results[c]["out"], dtype=np.float32)
    return acc.reshape(B, S, D)


# revision 26
# speedup vs baseline: 1.1155x; 1.1155x over previous
"""GQA attention prefill (B=2, S=1024, D=4096, H=32, KVH=8, HD=128) on 8 TRN2
NeuronCores.

Sharding: tensor-parallel over heads. Core c owns KV head c and query heads
4c..4c+3 (GQA groups align with cores), i.e. column-shards of wq/wk/wv and the
matching row-shard of wo. Each core computes its partial `attn_c @ wo_c`
([B*S, D]); the host sums the 8 partials.

Device inputs are host-pretiled into exact SBUF layouts so every DMA reads
fully contiguous per-partition lines (see _tile_* helpers):
  xh   [128, B*nkb, dc, 128]  bf16  activation half-chunks (contraction on
                                    partitions, 128 tokens per chunk)
  wq   [128, NH, dc, HD]  bf16  rope-permuted (even dims then odd, per head)
  wk   [128, dc, HD]      bf16  rope-permuted
  wv   [128, dc, HD]      bf16
  wo   [128, D/512, NH, 512] bf16
  sw   [128, 128]         bf16  identity (PE-transpose operand for V)
  c2   [128, S]   f32   rope cos table, duplicated across the two 64-row halves
  s2   [128, S]   f32   rope sin table, [-sin; +sin]
  mt   [128,128]  f32   causal diagonal-block additive mask * sqrt(HD)   (causal)
  mt   [S, S]     bf16  full transposed additive mask * sqrt(HD)         (general)

Q/K are produced transposed ([d, tok]) straight out of the projection matmul;
scores are computed transposed ([k, q]) so P^T feeds wide-N PV matmuls
(nested causal ranges) with no transposes anywhere. Softmax denominators are
built OFF the PE: a DVE pairwise add-tree folds the nkb k-blocks of P^T into
one [128, s] tile, a single GpSimd partition_all_reduce produces the
broadcast denominators, and a DVE reciprocal feeds the PV normalization
multiply. V is projected transposed and PE-transposed back. Rope's even/odd
pairing becomes a contiguous partition-half swap by permuting the weight
columns; the swap is two SBUF->SBUF DMAs. Softmax skips the max-subtraction
(scores are O(10); exp accumulates in fp32).

Startup: the first matmul needs only wk + the first x half-chunk. All x and
projection-weight DMAs ride the sync (HWDGE) queue in exact consumption order
(wk, x0, wv, x1, wq0, x2, wq1, x3, wq2, wq3, x4..), while the small rope/mask
tables load concurrently on the gpsimd queue — so the PE starts ~13us in and
never starves while the remaining weights stream. The first four half-chunks'
projection units are emitted in a hand-crafted order matched to DMA arrival.
wo rides the (otherwise idle) scalar queue so oproj never waits behind
output-store DMAs.

Scheduling: all work is emitted as unit-closures and "zipped" — batch 1's
projections interleave batch 0's attention, batch 0's output projection
interleaves batch 1's attention — so the in-order PE queue always holds dense
matmuls while exp/softmax chains drain on the other engines. PSUM evictions
are emitted one projection late to avoid head-of-line FIFO blocking.
"""

import math
from contextlib import ExitStack

import numpy as np
import ml_dtypes

import concourse.bass as bass
import concourse.bass_isa as bass_isa
import concourse.mybir as mybir
import concourse.tile as tile
from concourse import bacc
from concourse.bass_utils import run_bass_kernel_spmd

BF16 = mybir.dt.bfloat16
F32 = mybir.dt.float32
NPBF16 = ml_dtypes.bfloat16

B, S, D, H, KVH, HD = 2, 1024, 4096, 32, 8, 128
NCORES = 8
NH = H // NCORES          # 4 query heads per core
DC = D // 128             # 32 contraction chunks
SQ = 1.0 / math.sqrt(HD)


def _chunks(q0, qend, step=512):
    qs = q0
    while qs < qend:
        nq = min(step, qend - qs)
        yield qs, nq
        qs += nq


def build_program(causal, s=S, d=D):
    """Build the per-core SPMD program. s/d are overridable for sim tests."""
    dc = d // 128
    nkb = s // 128            # number of 128-wide key/query/token blocks per batch
    qcols = NH * HD

    # pT packing offsets: causal keeps only k-block ki's valid q range [128ki, s)
    if causal:
        q0s = [ki * 128 for ki in range(nkb)]
    else:
        q0s = [0] * nkb
    offs, acc = [], 0
    for ki in range(nkb):
        offs.append(acc)
        acc += s - q0s[ki]
    pt_len = acc

    nc = bacc.Bacc(
        "TRN2",
        target_bir_lowering=False,
        debug=False,
        enable_asserts=False,
        num_devices=1,
    )
    # all inputs are host-pretiled into the exact SBUF layouts, so every DMA
    # below reads fully contiguous per-partition lines
    xh = nc.dram_tensor("xh", [128, B * nkb, dc, 128], BF16, kind="ExternalInput").ap()
    wq = nc.dram_tensor("wq", [128, NH, dc, HD], BF16, kind="ExternalInput").ap()
    wk = nc.dram_tensor("wk", [128, dc, HD], BF16, kind="ExternalInput").ap()
    wv = nc.dram_tensor("wv", [128, dc, HD], BF16, kind="ExternalInput").ap()
    wo = nc.dram_tensor("wo", [128, d // 512, NH, 512], BF16, kind="ExternalInput").ap()
    sw = nc.dram_tensor("sw", [128, 128], BF16, kind="ExternalInput").ap()
    c2 = nc.dram_tensor("c2", [128, s], F32, kind="ExternalInput").ap()
    s2 = nc.dram_tensor("s2", [128, s], F32, kind="ExternalInput").ap()
    if causal:
        mt = nc.dram_tensor("mt", [128, 128], F32, kind="ExternalInput").ap()
    else:
        mt = nc.dram_tensor("mt", [s, s], BF16, kind="ExternalInput").ap()
    out = nc.dram_tensor("out", [B * s, d], BF16, kind="ExternalOutput").ap()

    with tile.TileContext(nc) as tc:
        with ExitStack() as ctx:
            const = ctx.enter_context(tc.tile_pool(name="const", bufs=1))
            xpool = ctx.enter_context(tc.tile_pool(name="xpool", bufs=4))
            wopool = ctx.enter_context(tc.tile_pool(name="wopool", bufs=2))
            qkv = ctx.enter_context(tc.tile_pool(name="qkv", bufs=2))
            ptp = ctx.enter_context(tc.tile_pool(name="ptp", bufs=2))
            rp = ctx.enter_context(tc.tile_pool(name="rp", bufs=3))
            small = ctx.enter_context(tc.tile_pool(name="small", bufs=2))
            oev = ctx.enter_context(tc.tile_pool(name="oev", bufs=2))
            psm = ctx.enter_context(tc.tile_pool(name="psm", bufs=4, space="PSUM"))
            psd = ctx.enter_context(tc.tile_pool(name="psd", bufs=1, space="PSUM"))
            pss = ctx.enter_context(tc.tile_pool(name="pss", bufs=2, space="PSUM"))

            # wk rides the sync queue FIRST (in front of the x stream): the
            # opening K-projection chain needs exactly wk + the first x half.
            # Split in two so the first 16 accumulation matmuls can start a
            # hair earlier.
            wk_sb = const.tile([128, dc, HD], BF16)
            nc.sync.dma_start(wk_sb[:, 0 : dc // 2, :], wk[:, 0 : dc // 2, :])
            nc.sync.dma_start(wk_sb[:, dc // 2 :, :], wk[:, dc // 2 :, :])
            # small rope/mask tables load concurrently on the gpsimd queue
            id_sb = const.tile([128, 128], BF16)
            nc.gpsimd.dma_start(id_sb[:], sw[:])
            c2_sb = const.tile([128, s], F32)
            nc.gpsimd.dma_start(c2_sb[:], c2[:])
            s2_sb = const.tile([128, s], F32)
            nc.gpsimd.dma_start(s2_sb[:], s2[:])
            if causal:
                mt_sb = const.tile([128, 128], F32)
                nc.gpsimd.dma_start(mt_sb[:], mt[:])
            else:
                mt_sb = const.tile([128, nkb, s], BF16)
                nc.gpsimd.dma_start(mt_sb[:], mt.rearrange("(kb p) q -> p kb q", p=128))
            # wv/wq are loaded lazily inside the first units that need them,
            # interleaved into the sync queue in exact consumption order.
            wv_sb = const.tile([128, dc, HD], BF16)
            wq_sb = const.tile([128, NH, dc, HD], BF16)
            loaded = set()
            ones_sb = const.tile([128, 1], BF16)
            nc.vector.memset(ones_sb[:], 1.0)

            def rope(pieces, tok0, w, out_slice):
                """pieces: per-128 psum tiles with raw projected Q/K columns
                (d-permuted). out = raw*c2 + halfswap(raw)*s2, written as bf16
                to out_slice. Evictions stay at 128-column PSUM grain; the
                swap / mul / add chain runs once per w columns."""
                raw = rp.tile([128, w], BF16, tag="raw", name=f"raw_{tok0}")
                for i, ps in enumerate(pieces):
                    nc.scalar.copy(raw[:, i * 128 : (i + 1) * 128], ps[:])
                swt = rp.tile([128, w], BF16, tag="swt", name=f"swt_{tok0}")
                nc.sync.dma_start(swt[0:64, :], raw[64:128, :])
                nc.sync.dma_start(swt[64:128, :], raw[0:64, :])
                t1 = rp.tile([128, w], F32, tag="t1", name=f"t1_{tok0}")
                nc.vector.tensor_mul(t1[:], swt[:], s2_sb[:, tok0 : tok0 + w])
                t2 = rp.tile([128, w], F32, tag="t2", name=f"t2_{tok0}")
                nc.vector.tensor_mul(t2[:], raw[:], c2_sb[:, tok0 : tok0 + w])
                nc.gpsimd.tensor_add(out_slice, t2[:], t1[:])

            def phase2(b, grain=1):
                """Stream x half-chunks, project Q/K/V for batch b. Returns
                the per-batch activation tiles. grain = number of 128-token
                halves per unit: batch 0 runs grain=1 for a fine-grained
                startup ramp; batch 1 runs grain=2 so the zipped filler units
                (and their rope/swap chains) stay coarse like the PE work."""
                qT_sb = qkv.tile([128, NH, s], BF16, tag="qT", name=f"qT_{b}")
                kT_sb = qkv.tile([128, s], BF16, tag="kT", name=f"kT_{b}")
                vT_sb = qkv.tile([128, s], BF16, tag="vT", name=f"vT_{b}")
                v_sb = qkv.tile([128, nkb, HD], BF16, tag="v", name=f"v_{b}")
                attnT_sb = qkv.tile([128, NH, s], BF16, tag="attnT", name=f"attnT_{b}")

                # evictions/rope are emitted one projection late, so each
                # engine's FIFO only sees work whose PSUM inputs are (nearly)
                # ready — avoids head-of-line blocking behind matmul chains.
                pending = []

                def flush(keep):
                    while len(pending) > keep:
                        kind, pieces, tok0_, w_ = pending.pop(0)
                        if kind == "k":
                            rope(pieces, tok0_, w_, kT_sb[:, tok0_ : tok0_ + w_])
                        elif kind.startswith("q"):
                            h = int(kind[1:])
                            rope(pieces, tok0_, w_, qT_sb[:, h, tok0_ : tok0_ + w_])
                        else:  # vt
                            for m2 in range(w_ // 128):
                                kb = tok0_ // 128 + m2
                                nc.vector.tensor_copy(
                                    vT_sb[:, kb * 128 : (kb + 1) * 128], pieces[m2][:]
                                )
                                vtp = pss.tile(
                                    [128, HD], BF16, tag="sm", name=f"vtp_{b}_{kb}"
                                )
                                nc.tensor.transpose(
                                    vtp[:], vT_sb[:, kb * 128 : (kb + 1) * 128], id_sb[:]
                                )
                                nc.scalar.copy(v_sb[:, kb, :], vtp[:])

                xcs = {}

                def load_x(js):
                    for j in js:
                        if j not in xcs:
                            xc = xpool.tile(
                                [128, dc, 128], BF16, tag="xc", name=f"xc_{b}_{j}"
                            )
                            nc.sync.dma_start(xc[:], xh[:, b * nkb + j, :, :])
                            xcs[j] = xc

                def proj(w_sb, js, nm):
                    # one PSUM tile + one clean start/stop accumulation chain
                    # per 128-token half (interleaving groups in a shared
                    # tile diverges on hardware)
                    pieces = []
                    for j in js:
                        ps = psm.tile([128, 128], F32, tag="mm", name=f"{nm}_{b}_{j}")
                        for c in range(dc):
                            nc.tensor.matmul(
                                ps[:],
                                w_sb[:, c, :],
                                xcs[j][:, c, :],
                                start=(c == 0),
                                stop=(c == dc - 1),
                            )
                        pieces.append(ps)
                    return pieces

                def mk_k(js):
                    def u():
                        load_x(js)
                        pending.append(("k", proj(wk_sb, js, "kps"), js[0] * 128, 128 * len(js)))
                        flush(1)
                    return u

                def mk_vt(js):
                    def u():
                        # V projection, transposed like K (wide-N matmuls),
                        # then PE-transposed back to natural [tok, d] layout
                        if b == 0 and "wv" not in loaded:
                            loaded.add("wv")
                            nc.sync.dma_start(wv_sb[:], wv[:])
                        pending.append(("vt", proj(wv_sb, js, "vps"), js[0] * 128, 128 * len(js)))
                        flush(1)
                    return u

                def mk_q(js, h):
                    def u():
                        if b == 0 and f"wq{h}" not in loaded:
                            loaded.add(f"wq{h}")
                            nc.sync.dma_start(wq_sb[:, h, :, :], wq[:, h, :, :])
                        pending.append((f"q{h}", proj(wq_sb[:, h], js, f"qps{h}"), js[0] * 128, 128 * len(js)))
                        flush(1)
                    return u

                mk = {"k": mk_k, "v": mk_vt}

                units = []
                if b == 0 and nkb >= 4:
                    # hand-crafted order for the first four half-chunks,
                    # matched to the DMA arrival order on the sync queue:
                    # wk | x0 | wv | x1 | wq0 | x2 | wq1 | x3 | wq2 | wq3 | x4..
                    crafted = [
                        (0, "k"), (0, "v"), (1, "k"), (1, "v"),
                        (0, "q0"), (1, "q0"), (2, "k"), (2, "v"),
                        (0, "q1"), (1, "q1"), (2, "q0"), (3, "k"), (3, "v"),
                        (2, "q1"), (0, "q2"), (1, "q2"), (0, "q3"), (1, "q3"),
                        (2, "q2"), (3, "q0"), (3, "q1"), (2, "q3"),
                        (3, "q2"), (3, "q3"),
                    ]
                    for j, kind in crafted:
                        if kind.startswith("q"):
                            units.append(mk_q([j], int(kind[1:])))
                        else:
                            units.append(mk[kind]([j]))
                    first = 4
                else:
                    first = 0
                for j0 in range(first, nkb, grain):
                    js = list(range(j0, min(j0 + grain, nkb)))
                    units.append(mk_k(js))
                    units.append(mk_vt(js))
                    for h in range(NH):
                        units.append(mk_q(js, h))
                units.append(lambda: flush(0))
                T = dict(qT=qT_sb, kT=kT_sb, vT=vT_sb, v=v_sb, attnT=attnT_sb)
                return T, units

            def attn(b, T):
                """Attention units for batch b, software-pipelined: PV of head
                h-1 is emitted after the scores of head h, so the softmax-
                denominator chain of head h-1 hides under head h's PE work."""
                qT_sb, kT_sb, v_sb, attnT_sb = T["qT"], T["kT"], T["v"], T["attnT"]
                stage1 = {}

                def attn_scores(h):
                    pT = ptp.tile([128, pt_len], BF16, tag="pt", name=f"pt_{b}_{h}")
                    sums = psd.tile([1, s], F32, tag="sums", bufs=1, name=f"sums_{b}_{h}")
                    for ki in range(nkb):
                        q0 = q0s[ki]
                        for qs_, nq in _chunks(q0, s):
                            sc = psm.tile([128, 512], F32, tag="mm", name=f"sc_{b}_{h}_{ki}_{qs_}")
                            nc.tensor.matmul(
                                sc[:, :nq],
                                kT_sb[:, ki * 128 : (ki + 1) * 128],
                                qT_sb[:, h, qs_ : qs_ + nq],
                                start=True,
                                stop=True,
                            )
                            if causal:
                                if qs_ == q0:  # diagonal block
                                    nc.vector.tensor_add(
                                        sc[:, 0:128], sc[:, 0:128], mt_sb[:]
                                    )
                            else:
                                nc.vector.tensor_add(
                                    sc[:, :nq], sc[:, :nq], mt_sb[:, ki, qs_ : qs_ + nq]
                                )
                            po = offs[ki] + qs_ - q0
                            nc.scalar.activation(
                                pT[:, po : po + nq],
                                sc[:, :nq],
                                mybir.ActivationFunctionType.Exp,
                                scale=SQ,
                            )
                            # denominators accumulate in PSUM across ki; the
                            # causal q-ranges nest, so ki==0 (full range)
                            # starts the group for every column. Pieces are
                            # split at 512-column boundaries so no matmul
                            # output crosses a PSUM bank.
                            a0 = qs_
                            while a0 < qs_ + nq:
                                a1 = min((a0 // 512 + 1) * 512, qs_ + nq)
                                nc.tensor.matmul(
                                    sums[0:1, a0:a1],
                                    ones_sb[:],
                                    pT[:, po + a0 - qs_ : po + a1 - qs_],
                                    start=(ki == 0),
                                    stop=(ki == nkb - 1),
                                    skip_group_check=True,
                                )
                                a0 = a1
                    # denominator chain, split into <=512 column pieces so each
                    # serial stage is short and pieces pipeline across engines
                    nhalf = (s + 511) // 512
                    width = s // nhalf
                    rbrs = []
                    for hs in range(nhalf):
                        ssb = small.tile([1, width], F32, tag="ssb", bufs=4, name=f"ssb_{b}_{h}_{hs}")
                        nc.scalar.copy(ssb[0:1, :], sums[0:1, hs * width : (hs + 1) * width])
                        rb = small.tile([128, width], F32, tag="rb", bufs=4, name=f"rb_{b}_{h}_{hs}")
                        nc.gpsimd.partition_broadcast(rb[:], ssb[0:1, :])
                        rbr = small.tile([128, width], F32, tag="rbr", bufs=4, name=f"rbr_{b}_{h}_{hs}")
                        nc.vector.reciprocal_approx_fast(rbr[:], rb[:])
                        rbrs.append(rbr)
                    return pT, rbrs, width

                def attn_pv(h):
                    # wide-N PV: per q-chunk, each k-block contributes one
                    # matmul over its (nested) valid q range, accumulating in
                    # PSUM — ki==0 always covers the whole chunk, so it opens
                    # the group for every column.
                    pT, rbrs, width = stage1.pop(h)
                    for ci, (qs0, w) in enumerate(_chunks(0, s)):
                        o_ps = psm.tile([128, 512], F32, tag="mm", name=f"ops_{b}_{h}_{ci}")
                        kis = [
                            k for k in range(nkb) if (not causal) or q0s[k] < qs0 + w
                        ]
                        for j, ki in enumerate(kis):
                            qlo = max(q0s[ki], qs0)
                            nc.tensor.matmul(
                                o_ps[:, qlo - qs0 : w],
                                v_sb[:, ki, :],
                                pT[:, offs[ki] + qlo - q0s[ki] : offs[ki] + qs0 + w - q0s[ki]],
                                start=(j == 0),
                                stop=(j == len(kis) - 1),
                                skip_group_check=True,
                            )
                        nc.vector.tensor_mul(
                            attnT_sb[:, h, qs0 : qs0 + w],
                            o_ps[:, :w],
                            rbrs[qs0 // width][:, qs0 % width : qs0 % width + w],
                        )

                units = []
                for h in range(NH):
                    units.append(lambda h=h: stage1.__setitem__(h, attn_scores(h)))
                    if h > 0:
                        units.append(lambda h=h: attn_pv(h - 1))
                units.append(lambda: attn_pv(NH - 1))
                return units

            def oproj(b, T):
                """Output projection units (partial over this core's wo rows).
                wo loads ride the (idle) scalar queue so they never queue
                behind output-store DMAs. A one-element write gated on the
                batch's LAST qT rope pins each load's readiness past the
                startup DMA crunch — the Tile list scheduler would otherwise
                hoist these dependency-free loads to t=0 where they steal
                bandwidth from the opening wk/x transfers."""
                attnT_sb = T["attnT"]
                qT_sb = T["qT"]
                wo_nbs = {}

                def mk(nb, tp):
                    def u():
                        if tp == 0:
                            wo_nb = wopool.tile(
                                [128, NH, 512], BF16, tag="wo", name=f"wo_{b}_{nb}"
                            )
                            nc.vector.tensor_copy(
                                wo_nb[0:1, 0, 0:1], qT_sb[0:1, NH - 1, s - 1 : s]
                            )
                            nc.scalar.dma_start(wo_nb[:], wo[:, nb, :, :])
                            wo_nbs[nb] = wo_nb
                        ot = oev.tile([128, 2, 512], BF16, tag="ot", bufs=4, name=f"ot_{b}_{nb}_{tp}")
                        for half in range(2):
                            tbk = tp * 2 + half
                            o2 = psm.tile([128, 512], F32, tag="mm", name=f"o2_{b}_{nb}_{tbk}")
                            for h in range(NH):
                                nc.tensor.matmul(
                                    o2[:],
                                    attnT_sb[:, h, tbk * 128 : (tbk + 1) * 128],
                                    wo_nbs[nb][:, h, :],
                                    start=(h == 0),
                                    stop=(h == NH - 1),
                                )
                            if half == 0:
                                nc.scalar.copy(ot[:, half, :], o2[:])
                            else:
                                nc.vector.tensor_copy(ot[:, half, :], o2[:])
                        (nc.sync if tp % 2 == 0 else nc.gpsimd).dma_start(
                            out[
                                b * s + tp * 256 : b * s + (tp + 1) * 256,
                                nb * 512 : (nb + 1) * 512,
                            ].rearrange("(rh p) n -> p rh n", p=128),
                            ot[:],
                        )
                    return u

                return [mk(nb, tp) for nb in range(d // 512) for tp in range(nkb // 2)]

            def zip_emit(primary, filler):
                """Emit primary units with filler units woven between them, so
                the in-order PE queue always has dense matmul work to run
                while the primary's cross-engine chains (exp/softmax) drain."""
                k = max(1, len(filler) // max(1, len(primary)))
                fi = 0
                for i, u in enumerate(primary):
                    if i == len(primary) - 1:
                        # drain fillers before the last primary: its softmax
                        # chain holds PSUM slots and would starve them
                        while fi < len(filler):
                            filler[fi]()
                            fi += 1
                    u()
                    for _ in range(k):
                        if fi < len(filler):
                            filler[fi]()
                            fi += 1
                while fi < len(filler):
                    filler[fi]()
                    fi += 1

            # batch 0 projections run alone; batch 0 attention is interleaved
            # with batch 1 projections; batch 1 attention with batch 0 output
            # projection; batch 1 output projection drains at the end.
            T0, p20 = phase2(0, grain=2)
            for u in p20:
                u()
            a0 = attn(0, T0)
            if B > 1:
                T1, p21 = phase2(1, grain=2)
                zip_emit(a0, p21)
                a1 = attn(1, T1)
                o0 = oproj(0, T0)
                zip_emit(a1, o0)
                for u in oproj(1, T1):
                    u()
            else:
                for u in a0:
                    u()
                for u in oproj(0, T0):
                    u()
    nc.compile()
    return nc


# ---------------------------------------------------------------------------
# host side
# ---------------------------------------------------------------------------

_PERM = np.concatenate([np.arange(0, HD, 2), np.arange(1, HD, 2)])
_CACHE = {}


def _tile_xh(x, s=S, d=D):
    """[B, s, d] f32 -> [128, B*nkb, dc, 128] bf16 (SBUF half-chunk layout)."""
    nkb, dc = s // 128, d // 128
    t = x.reshape(B, nkb, 128, dc, 128).transpose(4, 0, 1, 3, 2)
    return np.ascontiguousarray(t.reshape(128, B * nkb, dc, 128)).astype(NPBF16)


def _tile_wq(w, d=D):
    """[d, NH*HD] f32 (already rope-permuted) -> [128, NH, dc, HD] bf16."""
    dc = d // 128
    t = w.reshape(dc, 128, NH, HD).transpose(1, 2, 0, 3)
    return np.ascontiguousarray(t).astype(NPBF16)


def _tile_wkv(w, d=D):
    """[d, HD] f32 -> [128, dc, HD] bf16."""
    dc = d // 128
    return np.ascontiguousarray(w.reshape(dc, 128, HD).transpose(1, 0, 2)).astype(NPBF16)


def _tile_wo(w, d=D):
    """[NH*HD, d] f32 -> [128, d//512, NH, 512] bf16."""
    t = w.reshape(NH, 128, d // 512, 512).transpose(1, 2, 0, 3)
    return np.ascontiguousarray(t).astype(NPBF16)


def _get_program(causal):
    if causal not in _CACHE:
        _CACHE[causal] = build_program(causal)
    return _CACHE[causal]


def _is_causal(mask):
    iu = np.triu_indices(S, 1)
    il = np.tril_indices(S)
    return bool(np.all(mask[il] == 0.0) and np.all(mask[iu] < -1e8))


def make_in_maps(x, cos, sin, mask, wq, wk, wv, wo, causal):
    x = np.asarray(x, dtype=np.float32)
    cos = np.asarray(cos, dtype=np.float32)
    sin = np.asarray(sin, dtype=np.float32)
    mask = np.asarray(mask, dtype=np.float32)
    wq = np.asarray(wq, dtype=np.float32)
    wk = np.asarray(wk, dtype=np.float32)
    wv = np.asarray(wv, dtype=np.float32)
    wo = np.asarray(wo, dtype=np.float32)

    xh = _tile_xh(x)
    c2 = np.ascontiguousarray(np.concatenate([cos.T, cos.T], 0)).astype(np.float32)
    s2 = np.ascontiguousarray(np.concatenate([-sin.T, sin.T], 0)).astype(np.float32)
    swm = np.eye(128, dtype=np.float32).astype(NPBF16)  # transpose identity
    if causal:
        mt = np.ascontiguousarray(mask[:128, :128].T * math.sqrt(HD)).astype(np.float32)
    else:
        mt = np.ascontiguousarray(mask.T * math.sqrt(HD)).astype(NPBF16)

    in_maps = []
    for c in range(NCORES):
        wq_c = wq[:, c * NH * HD : (c + 1) * NH * HD].reshape(D, NH, HD)[:, :, _PERM]
        wq_c = _tile_wq(wq_c.reshape(D, NH * HD))
        wk_c = _tile_wkv(np.ascontiguousarray(wk[:, c * HD : (c + 1) * HD][:, _PERM]))
        wv_c = _tile_wkv(np.ascontiguousarray(wv[:, c * HD : (c + 1) * HD]))
        wo_c = _tile_wo(np.ascontiguousarray(wo[c * NH * HD : (c + 1) * NH * HD, :]))
        in_maps.append(
            {
                "xh": xh,
                "wq": wq_c,
                "wk": wk_c,
                "wv": wv_c,
                "wo": wo_c,
                "sw": swm,
                "c2": c2,
                "s2": s2,
                "mt": mt,
            }
        )
    return in_maps


def run(in_maps, causal, **kwargs):
    nc = _get_program(causal)
    return run_bass_kernel_spmd(nc, in_maps, core_ids=list(range(NCORES)), **kwargs)


def kernel(x, start_pos, cos, sin, mask, wq, wk, wv, wo):
    mask = np.asarray(mask, dtype=np.float32)
    causal = _is_causal(mask)
    in_maps = make_in_maps(x, cos, sin, mask, wq, wk, wv, wo, causal)
    res = run(in_maps, causal)
    acc = np.zeros((B * S, D), dtype=np.float32)
    for c in range(NCORES):
        acc += np.asarray(res.results[c]["out"], dtype=np.float32)
    return acc.reshape(B, S, D)


# revision 31
# speedup vs baseline: 1.1174x; 1.0018x over previous
"""GQA attention prefill (B=2, S=1024, D=4096, H=32, KVH=8, HD=128) on 8 TRN2
NeuronCores.

Sharding: tensor-parallel over heads. Core c owns KV head c and query heads
4c..4c+3 (GQA groups align with cores), i.e. column-shards of wq/wk/wv and the
matching row-shard of wo. Each core computes its partial `attn_c @ wo_c`
([B*S, D]); the host sums the 8 partials.

Device inputs are host-pretiled into exact SBUF layouts so every DMA reads
fully contiguous per-partition lines (see _tile_* helpers):
  xh   [128, B*nkb, dc, 128]  bf16  activation half-chunks (contraction on
                                    partitions, 128 tokens per chunk)
  wq   [128, NH, dc, HD]  bf16  rope-permuted (even dims then odd, per head)
  wk   [128, dc, HD]      bf16  rope-permuted
  wv   [128, dc, HD]      bf16
  wo   [128, D/512, NH, 512] bf16
  sw   [128, 128]         bf16  identity (PE-transpose operand for V)
  c2   [128, S]   f32   rope cos table, duplicated across the two 64-row halves
  s2   [128, S]   f32   rope sin table, [-sin; +sin]
  mt   [128,128]  f32   causal diagonal-block additive mask * sqrt(HD)   (causal)
  mt   [S, S]     bf16  full transposed additive mask * sqrt(HD)         (general)

Q/K are produced transposed ([d, tok]) straight out of the projection matmul;
scores are computed transposed ([k, q]) so P^T feeds wide-N PV matmuls
(nested causal ranges) with no transposes anywhere. Softmax denominators are
built OFF the PE: a DVE pairwise add-tree folds the nkb k-blocks of P^T into
one [128, s] tile, a single GpSimd partition_all_reduce produces the
broadcast denominators, and a DVE reciprocal feeds the PV normalization
multiply. V is projected transposed and PE-transposed back. Rope's even/odd
pairing becomes a contiguous partition-half swap by permuting the weight
columns; the swap is two SBUF->SBUF DMAs. Softmax skips the max-subtraction
(scores are O(10); exp accumulates in fp32).

Startup: the first matmul needs only wk + the first x half-chunk. All x and
projection-weight DMAs ride the sync (HWDGE) queue in exact consumption order
(wk, x0, wv, x1, wq0, x2, wq1, x3, wq2, wq3, x4..), while the small rope/mask
tables load concurrently on the gpsimd queue — so the PE starts ~13us in and
never starves while the remaining weights stream. The first four half-chunks'
projection units are emitted in a hand-crafted order matched to DMA arrival.
wo rides the (otherwise idle) scalar queue so oproj never waits behind
output-store DMAs.

Scheduling: all work is emitted as unit-closures and "zipped" — batch 1's
projections interleave batch 0's attention, batch 0's output projection
interleaves batch 1's attention — so the in-order PE queue always holds dense
matmuls while exp/softmax chains drain on the other engines. PSUM evictions
are emitted one projection late to avoid head-of-line FIFO blocking.
"""

import math
from contextlib import ExitStack

import numpy as np
import ml_dtypes

import concourse.bass as bass
import concourse.bass_isa as bass_isa
import concourse.mybir as mybir
import concourse.tile as tile
from concourse import bacc
from concourse.bass_utils import run_bass_kernel_spmd

BF16 = mybir.dt.bfloat16
F32 = mybir.dt.float32
NPBF16 = ml_dtypes.bfloat16

B, S, D, H, KVH, HD = 2, 1024, 4096, 32, 8, 128
NCORES = 8
NH = H // NCORES          # 4 query heads per core
DC = D // 128             # 32 contraction chunks
SQ = 1.0 / math.sqrt(HD)


def _chunks(q0, qend, step=512):
    qs = q0
    while qs < qend:
        nq = min(step, qend - qs)
        yield qs, nq
        qs += nq


def build_program(causal, s=S, d=D):
    """Build the per-core SPMD program. s/d are overridable for sim tests."""
    dc = d // 128
    nkb = s // 128            # number of 128-wide key/query/token blocks per batch
    qcols = NH * HD

    # pT packing offsets: causal keeps only k-block ki's valid q range [128ki, s)
    if causal:
        q0s = [ki * 128 for ki in range(nkb)]
    else:
        q0s = [0] * nkb
    offs, acc = [], 0
    for ki in range(nkb):
        offs.append(acc)
        acc += s - q0s[ki]
    pt_len = acc

    nc = bacc.Bacc(
        "TRN2",
        target_bir_lowering=False,
        debug=False,
        enable_asserts=False,
        num_devices=1,
    )
    # all inputs are host-pretiled into the exact SBUF layouts, so every DMA
    # below reads fully contiguous per-partition lines
    xh = nc.dram_tensor("xh", [128, B * nkb, dc, 128], BF16, kind="ExternalInput").ap()
    wq = nc.dram_tensor("wq", [128, NH, dc, HD], BF16, kind="ExternalInput").ap()
    wk = nc.dram_tensor("wk", [128, dc, HD], BF16, kind="ExternalInput").ap()
    wv = nc.dram_tensor("wv", [128, dc, HD], BF16, kind="ExternalInput").ap()
    wo = nc.dram_tensor("wo", [128, d // 512, NH, 512], BF16, kind="ExternalInput").ap()
    sw = nc.dram_tensor("sw", [128, 128], BF16, kind="ExternalInput").ap()
    c2 = nc.dram_tensor("c2", [128, s], F32, kind="ExternalInput").ap()
    s2 = nc.dram_tensor("s2", [128, s], F32, kind="ExternalInput").ap()
    if causal:
        mt = nc.dram_tensor("mt", [128, 128], F32, kind="ExternalInput").ap()
    else:
        mt = nc.dram_tensor("mt", [s, s], BF16, kind="ExternalInput").ap()
    out = nc.dram_tensor("out", [B * s, d], BF16, kind="ExternalOutput").ap()

    with tile.TileContext(nc) as tc:
        with ExitStack() as ctx:
            const = ctx.enter_context(tc.tile_pool(name="const", bufs=1))
            xpool = ctx.enter_context(tc.tile_pool(name="xpool", bufs=4))
            wopool = ctx.enter_context(tc.tile_pool(name="wopool", bufs=2))
            qkv = ctx.enter_context(tc.tile_pool(name="qkv", bufs=2))
            ptp = ctx.enter_context(tc.tile_pool(name="ptp", bufs=2))
            rp = ctx.enter_context(tc.tile_pool(name="rp", bufs=3))
            small = ctx.enter_context(tc.tile_pool(name="small", bufs=2))
            oev = ctx.enter_context(tc.tile_pool(name="oev", bufs=2))
            psm = ctx.enter_context(tc.tile_pool(name="psm", bufs=4, space="PSUM"))
            psd = ctx.enter_context(tc.tile_pool(name="psd", bufs=1, space="PSUM"))
            pss = ctx.enter_context(tc.tile_pool(name="pss", bufs=2, space="PSUM"))

            # wk rides the sync queue FIRST (in front of the x stream): the
            # opening K-projection chain needs exactly wk + the first x half.
            # Split in two so the first 16 accumulation matmuls can start a
            # hair earlier.
            wk_sb = const.tile([128, dc, HD], BF16)
            nc.sync.dma_start(wk_sb[:, 0 : dc // 2, :], wk[:, 0 : dc // 2, :])
            # (second wk half is emitted inside the first K unit, AFTER the
            # first x half-chunk, so the opening matmuls start ~1.4MB into
            # the sync stream instead of 2MB)
            # small tables load on the gpsimd queue; the big rope tables are
            # deferred to first use (see rope()) to keep the opening x/wk
            # transfers at full bandwidth
            id_sb = const.tile([128, 128], BF16)
            nc.gpsimd.dma_start(id_sb[:], sw[:])
            c2_sb = const.tile([128, s], F32)
            s2_sb = const.tile([128, s], F32)
            if causal:
                mt_sb = const.tile([128, 128], F32)
                nc.gpsimd.dma_start(mt_sb[:], mt[:])
            else:
                mt_sb = const.tile([128, nkb, s], BF16)
                nc.gpsimd.dma_start(mt_sb[:], mt.rearrange("(kb p) q -> p kb q", p=128))
            # wv/wq are loaded lazily inside the first units that need them,
            # interleaved into the sync queue in exact consumption order.
            wv_sb = const.tile([128, dc, HD], BF16)
            wq_sb = const.tile([128, NH, dc, HD], BF16)
            loaded = set()
            ones_sb = const.tile([128, 1], BF16)
            nc.vector.memset(ones_sb[:], 1.0)

            def rope(pieces, tok0, w, out_slice):
                """pieces: per-128 psum tiles with raw projected Q/K columns
                (d-permuted). out = raw*c2 + halfswap(raw)*s2, written as bf16
                to out_slice. Evictions stay at 128-column PSUM grain; the
                swap / mul / add chain runs once per w columns."""
                raw = rp.tile([128, w], BF16, tag="raw", name=f"raw_{tok0}")
                for i, ps in enumerate(pieces):
                    nc.scalar.copy(raw[:, i * 128 : (i + 1) * 128], ps[:])
                if "c2" not in loaded:
                    # lazy rope-table loads, write-after-write gated on the
                    # first rope's eviction so the scheduler cannot hoist
                    # them into the opening wk/x DMA window
                    loaded.add("c2")
                    nc.vector.tensor_copy(c2_sb[0:1, 0:1], raw[0:1, 0:1])
                    nc.gpsimd.dma_start(c2_sb[:], c2[:])
                    nc.vector.tensor_copy(s2_sb[0:1, 0:1], raw[0:1, 0:1])
                    nc.gpsimd.dma_start(s2_sb[:], s2[:])
                swt = rp.tile([128, w], BF16, tag="swt", name=f"swt_{tok0}")
                nc.sync.dma_start(swt[0:64, :], raw[64:128, :])
                nc.sync.dma_start(swt[64:128, :], raw[0:64, :])
                t1 = rp.tile([128, w], F32, tag="t1", name=f"t1_{tok0}")
                nc.vector.tensor_mul(t1[:], swt[:], s2_sb[:, tok0 : tok0 + w])
                t2 = rp.tile([128, w], F32, tag="t2", name=f"t2_{tok0}")
                nc.vector.tensor_mul(t2[:], raw[:], c2_sb[:, tok0 : tok0 + w])
                nc.gpsimd.tensor_add(out_slice, t2[:], t1[:])

            def phase2(b, grain=1):
                """Stream x half-chunks, project Q/K/V for batch b. Returns
                the per-batch activation tiles. grain = number of 128-token
                halves per unit: batch 0 runs grain=1 for a fine-grained
                startup ramp; batch 1 runs grain=2 so the zipped filler units
                (and their rope/swap chains) stay coarse like the PE work."""
                qT_sb = qkv.tile([128, NH, s], BF16, tag="qT", name=f"qT_{b}")
                kT_sb = qkv.tile([128, s], BF16, tag="kT", name=f"kT_{b}")
                vT_sb = qkv.tile([128, s], BF16, tag="vT", name=f"vT_{b}")
                v_sb = qkv.tile([128, nkb, HD], BF16, tag="v", name=f"v_{b}")
                attnT_sb = qkv.tile([128, NH, s], BF16, tag="attnT", name=f"attnT_{b}")

                # evictions/rope are emitted one projection late, so each
                # engine's FIFO only sees work whose PSUM inputs are (nearly)
                # ready — avoids head-of-line blocking behind matmul chains.
                pending = []

                def flush(keep):
                    while len(pending) > keep:
                        kind, pieces, tok0_, w_ = pending.pop(0)
                        if kind == "k":
                            rope(pieces, tok0_, w_, kT_sb[:, tok0_ : tok0_ + w_])
                        elif kind.startswith("q"):
                            h = int(kind[1:])
                            rope(pieces, tok0_, w_, qT_sb[:, h, tok0_ : tok0_ + w_])
                        else:  # vt
                            for m2 in range(w_ // 128):
                                kb = tok0_ // 128 + m2
                                nc.vector.tensor_copy(
                                    vT_sb[:, kb * 128 : (kb + 1) * 128], pieces[m2][:]
                                )
                                vtp = pss.tile(
                                    [128, HD], BF16, tag="sm", name=f"vtp_{b}_{kb}"
                                )
                                nc.tensor.transpose(
                                    vtp[:], vT_sb[:, kb * 128 : (kb + 1) * 128], id_sb[:]
                                )
                                nc.scalar.copy(v_sb[:, kb, :], vtp[:])

                xcs = {}

                def load_x(js):
                    for j in js:
                        if j not in xcs:
                            xc = xpool.tile(
                                [128, dc, 128], BF16, tag="xc", name=f"xc_{b}_{j}"
                            )
                            nc.sync.dma_start(xc[:], xh[:, b * nkb + j, :, :])
                            xcs[j] = xc

                def proj(w_sb, js, nm):
                    # one PSUM tile + one clean start/stop accumulation chain
                    # per 128-token half (interleaving groups in a shared
                    # tile diverges on hardware)
                    pieces = []
                    for j in js:
                        ps = psm.tile([128, 128], F32, tag="mm", name=f"{nm}_{b}_{j}")
                        for c in range(dc):
                            nc.tensor.matmul(
                                ps[:],
                                w_sb[:, c, :],
                                xcs[j][:, c, :],
                                start=(c == 0),
                                stop=(c == dc - 1),
                            )
                        pieces.append(ps)
                    return pieces

                def mk_k(js):
                    def u():
                        load_x(js)
                        if b == 0 and "wk2" not in loaded:
                            loaded.add("wk2")
                            nc.sync.dma_start(
                                wk_sb[:, dc // 2 :, :], wk[:, dc // 2 :, :]
                            )
                        pending.append(("k", proj(wk_sb, js, "kps"), js[0] * 128, 128 * len(js)))
                        flush(1)
                    return u

                def mk_vt(js):
                    def u():
                        # V projection, transposed like K (wide-N matmuls),
                        # then PE-transposed back to natural [tok, d] layout
                        if b == 0 and "wv" not in loaded:
                            loaded.add("wv")
                            nc.sync.dma_start(wv_sb[:], wv[:])
                        pending.append(("vt", proj(wv_sb, js, "vps"), js[0] * 128, 128 * len(js)))
                        flush(1)
                    return u

                def mk_q(js, h):
                    def u():
                        if b == 0 and f"wq{h}" not in loaded:
                            loaded.add(f"wq{h}")
                            nc.sync.dma_start(wq_sb[:, h, :, :], wq[:, h, :, :])
                        pending.append((f"q{h}", proj(wq_sb[:, h], js, f"qps{h}"), js[0] * 128, 128 * len(js)))
                        flush(1)
                    return u

                mk = {"k": mk_k, "v": mk_vt}

                units = []
                if b == 0 and nkb >= 4:
                    # hand-crafted order for the first four half-chunks,
                    # matched to the DMA arrival order on the sync queue:
                    # wk | x0 | wv | x1 | wq0 | x2 | wq1 | x3 | wq2 | wq3 | x4..
                    crafted = [
                        (0, "k"), (0, "v"), (1, "k"), (1, "v"),
                        (0, "q0"), (1, "q0"), (2, "k"), (2, "v"),
                        (0, "q1"), (1, "q1"), (2, "q0"), (3, "k"), (3, "v"),
                        (2, "q1"), (0, "q2"), (1, "q2"), (0, "q3"), (1, "q3"),
                        (2, "q2"), (3, "q0"), (3, "q1"), (2, "q3"),
                        (3, "q2"), (3, "q3"),
                    ]
                    for j, kind in crafted:
                        if kind.startswith("q"):
                            units.append(mk_q([j], int(kind[1:])))
                        else:
                            units.append(mk[kind]([j]))
                    first = 4
                else:
                    first = 0
                for j0 in range(first, nkb, grain):
                    js = list(range(j0, min(j0 + grain, nkb)))
                    units.append(mk_k(js))
                    units.append(mk_vt(js))
                    for h in range(NH):
                        units.append(mk_q(js, h))
                units.append(lambda: flush(0))
                T = dict(qT=qT_sb, kT=kT_sb, vT=vT_sb, v=v_sb, attnT=attnT_sb)
                return T, units

            def attn(b, T):
                """Attention units for batch b, software-pipelined: PV of head
                h-1 is emitted after the scores of head h, so the softmax-
                denominator chain of head h-1 hides under head h's PE work."""
                qT_sb, kT_sb, v_sb, attnT_sb = T["qT"], T["kT"], T["v"], T["attnT"]
                stage1 = {}

                def attn_scores(h):
                    pT = ptp.tile([128, pt_len], BF16, tag="pt", name=f"pt_{b}_{h}")
                    sums = psd.tile([1, s], F32, tag="sums", bufs=1, name=f"sums_{b}_{h}")
                    for ki in range(nkb):
                        q0 = q0s[ki]
                        for qs_, nq in _chunks(q0, s):
                            sc = psm.tile([128, 512], F32, tag="mm", name=f"sc_{b}_{h}_{ki}_{qs_}")
                            nc.tensor.matmul(
                                sc[:, :nq],
                                kT_sb[:, ki * 128 : (ki + 1) * 128],
                                qT_sb[:, h, qs_ : qs_ + nq],
                                start=True,
                                stop=True,
                            )
                            if causal:
                                if qs_ == q0:  # diagonal block
                                    nc.vector.tensor_add(
                                        sc[:, 0:128], sc[:, 0:128], mt_sb[:]
                                    )
                            else:
                                nc.vector.tensor_add(
                                    sc[:, :nq], sc[:, :nq], mt_sb[:, ki, qs_ : qs_ + nq]
                                )
                            po = offs[ki] + qs_ - q0
                            nc.scalar.activation(
                                pT[:, po : po + nq],
                                sc[:, :nq],
                                mybir.ActivationFunctionType.Exp,
                                scale=SQ,
                            )
                            # denominators accumulate in PSUM across ki; the
                            # causal q-ranges nest, so ki==0 (full range)
                            # starts the group for every column. Pieces are
                            # split at 512-column boundaries so no matmul
                            # output crosses a PSUM bank.
                            a0 = qs_
                            while a0 < qs_ + nq:
                                a1 = min((a0 // 512 + 1) * 512, qs_ + nq)
                                nc.tensor.matmul(
                                    sums[0:1, a0:a1],
                                    ones_sb[:],
                                    pT[:, po + a0 - qs_ : po + a1 - qs_],
                                    start=(ki == 0),
                                    stop=(ki == nkb - 1),
                                    skip_group_check=True,
                                )
                                a0 = a1
                    # denominator chain, split into <=512 column pieces so each
                    # serial stage is short and pieces pipeline across engines
                    nhalf = (s + 511) // 512
                    width = s // nhalf
                    rbrs = []
                    for hs in range(nhalf):
                        ssb = small.tile([1, width], F32, tag="ssb", bufs=4, name=f"ssb_{b}_{h}_{hs}")
                        nc.scalar.copy(ssb[0:1, :], sums[0:1, hs * width : (hs + 1) * width])
                        rb = small.tile([128, width], F32, tag="rb", bufs=4, name=f"rb_{b}_{h}_{hs}")
                        nc.gpsimd.partition_broadcast(rb[:], ssb[0:1, :])
                        rbr = small.tile([128, width], F32, tag="rbr", bufs=4, name=f"rbr_{b}_{h}_{hs}")
                        nc.vector.reciprocal_approx_fast(rbr[:], rb[:])
                        rbrs.append(rbr)
                    return pT, rbrs, width

                def attn_pv(h):
                    # wide-N PV: per q-chunk, each k-block contributes one
                    # matmul over its (nested) valid q range, accumulating in
                    # PSUM — ki==0 always covers the whole chunk, so it opens
                    # the group for every column.
                    pT, rbrs, width = stage1.pop(h)
                    for ci, (qs0, w) in enumerate(_chunks(0, s)):
                        o_ps = psm.tile([128, 512], F32, tag="mm", name=f"ops_{b}_{h}_{ci}")
                        kis = [
                            k for k in range(nkb) if (not causal) or q0s[k] < qs0 + w
                        ]
                        for j, ki in enumerate(kis):
                            qlo = max(q0s[ki], qs0)
                            nc.tensor.matmul(
                                o_ps[:, qlo - qs0 : w],
                                v_sb[:, ki, :],
                                pT[:, offs[ki] + qlo - q0s[ki] : offs[ki] + qs0 + w - q0s[ki]],
                                start=(j == 0),
                                stop=(j == len(kis) - 1),
                                skip_group_check=True,
                            )
                        nc.vector.tensor_mul(
                            attnT_sb[:, h, qs0 : qs0 + w],
                            o_ps[:, :w],
                            rbrs[qs0 // width][:, qs0 % width : qs0 % width + w],
                        )

                units = []
                for h in range(NH):
                    units.append(lambda h=h: stage1.__setitem__(h, attn_scores(h)))
                    if h > 0:
                        units.append(lambda h=h: attn_pv(h - 1))
                units.append(lambda: attn_pv(NH - 1))
                return units

            def oproj(b, T):
                """Output projection units (partial over this core's wo rows).
                wo loads ride the (idle) scalar queue so they never queue
                behind output-store DMAs. A one-element write gated on the
                batch's LAST qT rope pins each load's readiness past the
                startup DMA crunch — the Tile list scheduler would otherwise
                hoist these dependency-free loads to t=0 where they steal
                bandwidth from the opening wk/x transfers."""
                attnT_sb = T["attnT"]
                qT_sb = T["qT"]
                wo_nbs = {}

                def mk(nb, tp):
                    def u():
                        if tp == 0:
                            wo_nb = wopool.tile(
                                [128, NH, 512], BF16, tag="wo", name=f"wo_{b}_{nb}"
                            )
                            nc.vector.tensor_copy(
                                wo_nb[0:1, 0, 0:1], qT_sb[0:1, NH - 1, s - 1 : s]
                            )
                            nc.scalar.dma_start(wo_nb[:], wo[:, nb, :, :])
                            wo_nbs[nb] = wo_nb
                        ot = oev.tile([128, 2, 512], BF16, tag="ot", bufs=6, name=f"ot_{b}_{nb}_{tp}")
                        for half in range(2):
                            tbk = tp * 2 + half
                            o2 = psm.tile([128, 512], F32, tag="mm", name=f"o2_{b}_{nb}_{tbk}")
                            for h in range(NH):
                                nc.tensor.matmul(
                                    o2[:],
                                    attnT_sb[:, h, tbk * 128 : (tbk + 1) * 128],
                                    wo_nbs[nb][:, h, :],
                                    start=(h == 0),
                                    stop=(h == NH - 1),
                                )
                            if half == 0:
                                nc.scalar.copy(ot[:, half, :], o2[:])
                            else:
                                nc.vector.tensor_copy(ot[:, half, :], o2[:])
                        (nc.sync if tp % 2 == 0 else nc.gpsimd).dma_start(
                            out[
                                b * s + tp * 256 : b * s + (tp + 1) * 256,
                                nb * 512 : (nb + 1) * 512,
                            ].rearrange("(rh p) n -> p rh n", p=128),
                            ot[:],
                        )
                    return u

                return [mk(nb, tp) for nb in range(d // 512) for tp in range(nkb // 2)]

            def zip_emit(primary, filler):
                """Emit primary units with filler units woven between them, so
                the in-order PE queue always has dense matmul work to run
                while the primary's cross-engine chains (exp/softmax) drain."""
                k = max(1, len(filler) // max(1, len(primary)))
                fi = 0
                for i, u in enumerate(primary):
                    u()
                    for _ in range(k):
                        if fi < len(filler):
                            filler[fi]()
                            fi += 1
                # remaining fillers drain AFTER the last primary: draining
                # them first serializes their eviction/out-DMA chains in
                # front of the last primary's matmuls (measured ~6us PE gap
                # + a HAM re-throttle)
                while fi < len(filler):
                    filler[fi]()
                    fi += 1

            # batch 0 projections run alone; batch 0 attention is interleaved
            # with batch 1 projections; batch 1 attention with batch 0 output
            # projection; batch 1 output projection drains at the end.
            T0, p20 = phase2(0, grain=2)
            for u in p20:
                u()
            a0 = attn(0, T0)
            if B > 1:
                T1, p21 = phase2(1, grain=2)
                zip_emit(a0, p21)
                a1 = attn(1, T1)
                o0 = oproj(0, T0)
                zip_emit(a1, o0)
                for u in oproj(1, T1):
                    u()
            else:
                for u in a0:
                    u()
                for u in oproj(0, T0):
                    u()
    nc.compile()
    return nc


# ---------------------------------------------------------------------------
# host side
# ---------------------------------------------------------------------------

_PERM = np.concatenate([np.arange(0, HD, 2), np.arange(1, HD, 2)])
_CACHE = {}


def _tile_xh(x, s=S, d=D):
    """[B, s, d] f32 -> [128, B*nkb, dc, 128] bf16 (SBUF half-chunk layout)."""
    nkb, dc = s // 128, d // 128
    t = x.reshape(B, nkb, 128, dc, 128).transpose(4, 0, 1, 3, 2)
    return np.ascontiguousarray(t.reshape(128, B * nkb, dc, 128)).astype(NPBF16)


def _tile_wq(w, d=D):
    """[d, NH*HD] f32 (already rope-permuted) -> [128, NH, dc, HD] bf16."""
    dc = d // 128
    t = w.reshape(dc, 128, NH, HD).transpose(1, 2, 0, 3)
    return np.ascontiguousarray(t).astype(NPBF16)


def _tile_wkv(w, d=D):
    """[d, HD] f32 -> [128, dc, HD] bf16."""
    dc = d // 128
    return np.ascontiguousarray(w.reshape(dc, 128, HD).transpose(1, 0, 2)).astype(NPBF16)


def _tile_wo(w, d=D):
    """[NH*HD, d] f32 -> [128, d//512, NH, 512] bf16."""
    t = w.reshape(NH, 128, d // 512, 512).transpose(1, 2, 0, 3)
    return np.ascontiguousarray(t).astype(NPBF16)


def _get_program(causal):
    if causal not in _CACHE:
        _CACHE[causal] = build_program(causal)
    return _CACHE[causal]


def _is_causal(mask):
    iu = np.triu_indices(S, 1)
    il = np.tril_indices(S)
    return bool(np.all(mask[il] == 0.0) and np.all(mask[iu] < -1e8))


def make_in_maps(x, cos, sin, mask, wq, wk, wv, wo, causal):
    x = np.asarray(x, dtype=np.float32)
    cos = np.asarray(cos, dtype=np.float32)
    sin = np.asarray(sin, dtype=np.float32)
    mask = np.asarray(mask, dtype=np.float32)
    wq = np.asarray(wq, dtype=np.float32)
    wk = np.asarray(wk, dtype=np.float32)
    wv = np.asarray(wv, dtype=np.float32)
    wo = np.asarray(wo, dtype=np.float32)

    xh = _tile_xh(x)
    c2 = np.ascontiguousarray(np.concatenate([cos.T, cos.T], 0)).astype(np.float32)
    s2 = np.ascontiguousarray(np.concatenate([-sin.T, sin.T], 0)).astype(np.float32)
    swm = np.eye(128, dtype=np.float32).astype(NPBF16)  # transpose identity
    if causal:
        mt = np.ascontiguousarray(mask[:128, :128].T * math.sqrt(HD)).astype(np.float32)
    else:
        mt = np.ascontiguousarray(mask.T * math.sqrt(HD)).astype(NPBF16)

    in_maps = []
    for c in range(NCORES):
        wq_c = wq[:, c * NH * HD : (c + 1) * NH * HD].reshape(D, NH, HD)[:, :, _PERM]
        wq_c = _tile_wq(wq_c.reshape(D, NH * HD))
        wk_c = _tile_wkv(np.ascontiguousarray(wk[:, c * HD : (c + 1) * HD][:, _PERM]))
        wv_c = _tile_wkv(np.ascontiguousarray(wv[:, c * HD : (c + 1) * HD]))
        wo_c = _tile_wo(np.ascontiguousarray(wo[c * NH * HD : (c + 1) * NH * HD, :]))
        in_maps.append(
            {
                "xh": xh,
                "wq": wq_c,
                "wk": wk_c,
                "wv": wv_c,
                "wo": wo_c,
                "sw": swm,
                "c2": c2,
                "s2": s2,
                "mt": mt,
            }
        )
    return in_maps


def run(in_maps, causal, **kwargs):
    nc = _get_program(causal)
    return run_bass_kernel_spmd(nc, in_maps, core_ids=list(range(NCORES)), **kwargs)


def kernel(x, start_pos, cos, sin, mask, wq, wk, wv, wo):
    mask = np.asarray(mask, dtype=np.float32)
    causal = _is_causal(mask)
    in_maps = make_in_maps(x, cos, sin, mask, wq, wk, wv, wo, causal)
    res = run(in_maps, causal)
    acc = np.zeros((B * S, D), dtype=np.float32)
    for c in range(NCORES):
        acc += np.asarray(res.results[c]["out"], dtype=np.float32)
    return acc.reshape(B, S, D)
